# revision 1
# baseline (speedup 1.0000x reference)
"""GraphSAGE layer on 8 Trainium2 NeuronCores (Bass/Tile).

Sharding: data-parallel over the 50000 target nodes (6250 rows/core), feature
table + weights replicated. Per core: indirect-DMA gather of self + 25
neighbor feature rows, neighbor mean via DVE adds, PE transposes + matmuls
(out.T = W1 @ self.T + (W2/25) @ neighsum.T), ReLU+bias on ACT with fused
free-dim accumulation for BN stats, AllReduce of per-core (sum, sumsq),
BN apply + row L2-normalize on device, per-shard output written back.
"""
from contextlib import ExitStack

import numpy as np

import concourse.bacc as bacc
import concourse.bass as bass
import concourse.tile as tile
from concourse import mybir
from concourse.bass_utils import run_bass_kernel_spmd
from concourse.masks import make_identity

BN_EPS = 1e-5
NORM_EPS = 1e-6

N_CORES = 8
N_TOTAL = 50000          # target nodes
TABLE_ROWS = 200000
D = 128
S = 25                   # neighbors
SLOTS = S + 1            # self + neighbors
P = 128

_prog_cache = {}


def build_program(rows_per_core, table_rows, n_cores, n_total):
    n_tiles = (rows_per_core + P - 1) // P
    pad_rows = n_tiles * P

    nc = bacc.Bacc("TRN2", target_bir_lowering=False, num_devices=n_cores)
    f32 = mybir.dt.float32
    feat = nc.dram_tensor("features", [table_rows, D], f32, kind="ExternalInput")
    idx = nc.dram_tensor("idx", [P, n_tiles * SLOTS], mybir.dt.int32,
                         kind="ExternalInput")
    w1t = nc.dram_tensor("w1t", [D, D], f32, kind="ExternalInput")
    w2ts = nc.dram_tensor("w2ts", [D, D], f32, kind="ExternalInput")
    bvec = nc.dram_tensor("bvec", [D, 1], f32, kind="ExternalInput")
    gvec = nc.dram_tensor("gvec", [D, 1], f32, kind="ExternalInput")
    betav = nc.dram_tensor("betav", [D, 1], f32, kind="ExternalInput")
    out = nc.dram_tensor("out", [rows_per_core, D], f32, kind="ExternalOutput")

    ar_in = nc.dram_tensor("ar_in", [D, 2], f32)
    ar_out = nc.dram_tensor("ar_out", [D, 2], f32, addr_space="Shared")

    with tile.TileContext(nc) as tc:
        with ExitStack() as ctx:
            singles = ctx.enter_context(tc.tile_pool(name="singles", bufs=1))
            gpool = ctx.enter_context(tc.tile_pool(name="gpool", bufs=7))
            wpool = ctx.enter_context(tc.tile_pool(name="wpool", bufs=2))
            psum = ctx.enter_context(tc.tile_pool(name="psum", bufs=2,
                                                  space="PSUM"))
            psum2 = ctx.enter_context(tc.tile_pool(name="psum2", bufs=2,
                                                   space="PSUM"))

            idx_sb = singles.tile([P, n_tiles * SLOTS], mybir.dt.int32)
            nc.sync.dma_start(out=idx_sb[:], in_=idx[:])
            w1t_sb = singles.tile([D, D], f32)
            nc.sync.dma_start(out=w1t_sb[:], in_=w1t[:])
            w2ts_sb = singles.tile([D, D], f32)
            nc.sync.dma_start(out=w2ts_sb[:], in_=w2ts[:])
            b_sb = singles.tile([D, 1], f32)
            nc.sync.dma_start(out=b_sb[:], in_=bvec[:])
            g_sb = singles.tile([D, 1], f32)
            nc.sync.dma_start(out=g_sb[:], in_=gvec[:])
            beta_sb = singles.tile([D, 1], f32)
            nc.sync.dma_start(out=beta_sb[:], in_=betav[:])
            ident = singles.tile([P, P], f32)
            make_identity(nc, ident[:])

            zbuf = singles.tile([P, n_tiles, P], f32)       # pre-BN, [feat, row]
            sums = singles.tile([P, n_tiles], f32)
            sumsq = singles.tile([P, n_tiles], f32)

            # ---------------- Phase A: gather + matmul + relu ----------------
            for t in range(n_tiles):
                ga = gpool.tile([P, SLOTS, D], f32, tag="ga")
                for j in range(SLOTS):
                    nc.gpsimd.indirect_dma_start(
                        out=ga[:, j, :],
                        out_offset=None,
                        in_=feat[:, :],
                        in_offset=bass.IndirectOffsetOnAxis(
                            ap=idx_sb[:, t * SLOTS + j:t * SLOTS + j + 1],
                            axis=0),
                    )
                # neighbor sum: slots 1..25 (tree adds on DVE)
                s1 = wpool.tile([P, 12, D], f32, tag="s1")
                nc.vector.tensor_add(s1[:], ga[:, 1:13, :], ga[:, 13:25, :])
                s2 = wpool.tile([P, 6, D], f32, tag="s2")
                nc.vector.tensor_add(s2[:], s1[:, 0:6, :], s1[:, 6:12, :])
                s3 = wpool.tile([P, 3, D], f32, tag="s3")
                nc.vector.tensor_add(s3[:], s2[:, 0:3, :], s2[:, 3:6, :])
                agg = wpool.tile([P, D], f32, tag="agg")
                nc.vector.tensor_add(agg[:], s3[:, 0, :], s3[:, 1, :])
                nc.vector.tensor_add(agg[:], agg[:], s3[:, 2, :])
                nc.vector.tensor_add(agg[:], agg[:], ga[:, 25, :])

                # transposes via PE (SBUF -> PSUM), copy back on ACT
                pT = psum.tile([P, P], f32, tag="pT")
                nc.tensor.transpose(out=pT[:], in_=ga[:, 0, :], identity=ident[:])
                sT = wpool.tile([P, P], f32, tag="sT")
                nc.scalar.copy(out=sT[:], in_=pT[:])
                pT2 = psum.tile([P, P], f32, tag="pT2")
                nc.tensor.transpose(out=pT2[:], in_=agg[:], identity=ident[:])
                aT = wpool.tile([P, P], f32, tag="aT")
                nc.scalar.copy(out=aT[:], in_=pT2[:])

                # out.T = W1 @ sT + (W2/25) @ aT   (lhsT = W1T / W2Ts)
                mm = psum2.tile([P, P], f32, tag="mm")
                nc.tensor.matmul(mm[:], w1t_sb[:], sT[:], start=True, stop=False)
                nc.tensor.matmul(mm[:], w2ts_sb[:], aT[:], start=False, stop=True)

                # relu + bias (feat on partitions); accumulate BN stats over
                # valid rows only
                nv = min(P, rows_per_core - t * P)
                if nv == P:
                    nc.scalar.activation(
                        out=zbuf[:, t, :], in_=mm[:],
                        func=mybir.ActivationFunctionType.Relu,
                        bias=b_sb[:], scale=1.0,
                        accum_out=sums[:, t:t + 1])
                    dump = wpool.tile([P, P], f32, tag="dump")
                    nc.scalar.activation(
                        out=dump[:], in_=zbuf[:, t, :],
                        func=mybir.ActivationFunctionType.Square,
                        accum_out=sumsq[:, t:t + 1])
                else:
                    nc.scalar.activation(
                        out=zbuf[:, t, 0:nv], in_=mm[:, 0:nv],
                        func=mybir.ActivationFunctionType.Relu,
                        bias=b_sb[:], scale=1.0,
                        accum_out=sums[:, t:t + 1])
                    dump = wpool.tile([P, P], f32, tag="dump")
                    nc.scalar.activation(
                        out=dump[:, 0:nv], in_=zbuf[:, t, 0:nv],
                        func=mybir.ActivationFunctionType.Square,
                        accum_out=sumsq[:, t:t + 1])

            # ---------------- Phase B: global BN stats -----------------------
            gstat = singles.tile([P, 2], f32)
            nc.vector.tensor_reduce(out=gstat[:, 0:1], in_=sums[:],
                                    axis=mybir.AxisListType.X,
                                    op=mybir.AluOpType.add)
            nc.vector.tensor_reduce(out=gstat[:, 1:2], in_=sumsq[:],
                                    axis=mybir.AxisListType.X,
                                    op=mybir.AluOpType.add)
            nc.sync.dma_start(out=ar_in[:], in_=gstat[:])
            nc.gpsimd.collective_compute(
                "AllReduce", mybir.AluOpType.add,
                ins=[ar_in[:]],
                outs=[ar_out[:]],
                replica_groups=[list(range(n_cores))],
            )
            gg = singles.tile([P, 2], f32)
            nc.sync.dma_start(out=gg[:], in_=ar_out[:])

            inv_n = 1.0 / float(n_total)
            mu = singles.tile([P, 1], f32)
            nc.vector.tensor_scalar_mul(mu[:], gg[:, 0:1], inv_n)
            ex2 = singles.tile([P, 1], f32)
            nc.vector.tensor_scalar_mul(ex2[:], gg[:, 1:2], inv_n)
            var = singles.tile([P, 1], f32)
            nc.vector.tensor_mul(var[:], mu[:], mu[:])
            nc.vector.tensor_sub(var[:], ex2[:], var[:])
            nc.vector.tensor_scalar_add(var[:], var[:], BN_EPS)
            std = singles.tile([P, 1], f32)
            nc.scalar.sqrt(out=std[:], in_=var[:])
            rstd = singles.tile([P, 1], f32)
            nc.vector.reciprocal(out=rstd[:], in_=std[:])
            gp = singles.tile([P, 1], f32)
            nc.vector.tensor_mul(gp[:], g_sb[:], rstd[:])
            sh = singles.tile([P, 1], f32)
            nc.vector.tensor_mul(sh[:], mu[:], gp[:])
            nc.vector.tensor_sub(sh[:], beta_sb[:], sh[:])

            # ---------------- Phase C: BN apply + L2 normalize ---------------
            for t in range(n_tiles):
                nv = min(P, rows_per_core - t * P)
                bnz = wpool.tile([P, P], f32, tag="bnz")
                nc.vector.tensor_scalar(
                    out=bnz[:], in0=zbuf[:, t, :],
                    scalar1=gp[:], scalar2=sh[:],
                    op0=mybir.AluOpType.mult, op1=mybir.AluOpType.add)
                pT3 = psum.tile([P, P], f32, tag="pT3")
                nc.tensor.transpose(out=pT3[:], in_=bnz[:], identity=ident[:])
                yT = wpool.tile([P, P], f32, tag="yT")
                nc.scalar.copy(out=yT[:], in_=pT3[:])
                ysq = wpool.tile([P, P], f32, tag="ysq")
                n2 = wpool.tile([P, 1], f32, tag="n2")
                nc.scalar.activation(
                    out=ysq[:], in_=yT[:],
                    func=mybir.ActivationFunctionType.Square,
                    accum_out=n2[:])
                nrm = wpool.tile([P, 1], f32, tag="nrm")
                nc.scalar.sqrt(out=nrm[:], in_=n2[:])
                nc.vector.tensor_scalar_add(nrm[:], nrm[:], NORM_EPS)
                rn = wpool.tile([P, 1], f32, tag="rn")
                nc.vector.reciprocal(out=rn[:], in_=nrm[:])
                y = wpool.tile([P, P], f32, tag="y")
                nc.vector.tensor_scalar_mul(y[:], yT[:], rn[:])
                nc.sync.dma_start(out=out[t * P:t * P + nv, :], in_=y[0:nv, :])

    nc.compile()
    return nc


def _get_program(rows_per_core, table_rows, n_cores, n_total):
    key = (rows_per_core, table_rows, n_cores, n_total)
    if key not in _prog_cache:
        _prog_cache[key] = build_program(rows_per_core, table_rows, n_cores,
                                         n_total)
    return _prog_cache[key]


def kernel(features, self_idx, neigh_idx, W, b, gamma, beta):
    features = np.ascontiguousarray(np.asarray(features, dtype=np.float32))
    self_idx = np.asarray(self_idx).astype(np.int64)
    neigh_idx = np.asarray(neigh_idx).astype(np.int64)
    W = np.asarray(W, dtype=np.float32)
    n, s = neigh_idx.shape
    table_rows, d = features.shape
    n_cores = N_CORES
    rows_per_core = n // n_cores
    n_tiles = (rows_per_core + P - 1) // P
    pad_rows = n_tiles * P

    w1t = np.ascontiguousarray(W[:, :d].T)             # [d, d]
    w2ts = np.ascontiguousarray((W[:, d:] / float(s)).T)
    bvec = np.asarray(b, dtype=np.float32).reshape(d, 1).copy()
    gvec = np.asarray(gamma, dtype=np.float32).reshape(d, 1).copy()
    betav = np.asarray(beta, dtype=np.float32).reshape(d, 1).copy()

    # combined [n, 26] index matrix: slot 0 = self, 1..25 = neighbors
    allidx = np.concatenate([self_idx[:, None], neigh_idx], axis=1)

    nc = _get_program(rows_per_core, table_rows, n_cores, n)

    in_maps = []
    for c in range(n_cores):
        sl = allidx[c * rows_per_core:(c + 1) * rows_per_core]
        padded = np.zeros((pad_rows, SLOTS), dtype=np.int32)
        padded[:rows_per_core] = sl
        # [pad_rows, SLOTS] -> [n_tiles, P, SLOTS] -> [P, n_tiles, SLOTS]
        arr = padded.reshape(n_tiles, P, SLOTS).transpose(1, 0, 2)
        arr = np.ascontiguousarray(arr.reshape(P, n_tiles * SLOTS))
        in_maps.append({
            "features": features,
            "idx": arr,
            "w1t": w1t,
            "w2ts": w2ts,
            "bvec": bvec,
            "gvec": gvec,
            "betav": betav,
        })

    global _last_in_maps
    _last_in_maps = in_maps
    res = run_bass_kernel_spmd(nc, in_maps, core_ids=list(range(n_cores)))
    outp = np.concatenate([res.results[c]["out"] for c in range(n_cores)],
                          axis=0)
    return outp


_last_in_maps = None

